# revision 21
# baseline (speedup 1.0000x reference)
"""CausalMaskedLinear Trainium2 kernel.

y = x @ (W * mask).T + b, with mask a deterministic block-banded causal
pattern: output time-step block o (128 rows) attends to input blocks
j in [o-7, o] (TRI_BLOCK=8), 128 cols each.  Only 228 of the 1024
128x128 weight blocks are live.

Strategy: data-parallel over batch (8192/8 = 1024 rows per core),
weights/bias replicated.  Host packs x transposed ([in_feat, batch]) and
the live weight blocks transposed ([in, out] layout) so the device loop
is a pure stream of PSUM-accumulated matmuls:
    yT[o*128:, b] = sum_j WT_block(o,j).T @ xT_block(j)[:, b]   (+ bias)
x and W are fed in fp16 (max scale-relative error ~3e-4 for this
problem's distributions; values are well inside fp16 range), accumulated
in fp32 PSUM.  Bias is added during the PSUM->SBUF copy on the vector
engine; output stays fp32.  Output is produced transposed and restored
on host.
"""

import numpy as np

NUM_TIME_STEPS = 32
IN_FEAT = 128
OUT_FEAT = 128
TRI_BLOCK = 8
BATCH = 8192
N_CORES = 8
BC = BATCH // N_CORES  # batch rows per core

IN_SIZE = NUM_TIME_STEPS * IN_FEAT
OUT_SIZE = NUM_TIME_STEPS * OUT_FEAT

OG = 4  # output blocks per weight-DMA group
XG = 4  # input blocks per x-DMA chunk


def _band(o):
    return range(max(0, o - TRI_BLOCK + 1), o + 1)


N_BLOCKS = sum(len(_band(o)) for o in range(NUM_TIME_STEPS))  # 228
_KSTART = np.cumsum([0] + [len(_band(o)) for o in range(NUM_TIME_STEPS)])

_PROGRAM = None


def _build_program():
    import concourse.bacc as bacc
    import concourse.bass as bass
    import concourse.mybir as mybir
    import concourse.tile as tile

    f32 = mybir.dt.float32
    f16 = mybir.dt.float16

    nc = bacc.Bacc("TRN2", target_bir_lowering=False, debug=False,
                   enable_asserts=False)

    xT_d = nc.dram_tensor("xT", [128, NUM_TIME_STEPS * BC], f16,
                          kind="ExternalInput")
    wt_d = nc.dram_tensor("wt", [128, N_BLOCKS * 128], f16,
                          kind="ExternalInput")
    bias_d = nc.dram_tensor("bias_t", [128, NUM_TIME_STEPS], f32,
                            kind="ExternalInput")
    yT_d = nc.dram_tensor("yT", [NUM_TIME_STEPS, 128, BC], f32,
                          kind="ExternalOutput")

    NH = BC // 512  # moving-dim pieces per output tile

    with tile.TileContext(nc) as tc:
        with (
            tc.tile_pool(name="xp", bufs=NUM_TIME_STEPS) as xp,
            tc.tile_pool(name="wp", bufs=12) as wp,
            tc.tile_pool(name="op", bufs=4) as op,
            tc.tile_pool(name="bp", bufs=1) as bp,
            tc.tile_pool(name="psp", bufs=8, space=bass.MemorySpace.PSUM) as psp,
        ):
            # All load DMAs are issued from the Scalar engine (idle
            # otherwise, and its preamble retires ~2us before Sync's, so
            # the first tiles land sooner).  Output DMAs stay on Sync.
            # wp's bufs slot-limit flow-controls the weight prefetch depth.
            bias_t = bp.tile([128, NUM_TIME_STEPS], f32)
            nc.scalar.dma_start(bias_t[:], bias_d[:])

            # Pre-warm the PE while the first loads are in flight: HAM
            # un-throttles (1.2 -> 2.4 GHz) only after ~3.4us of sustained
            # activity, so burn the head DMA latency on dummy matmuls.
            warm_in = xp.tile([128, 512], f16, tag="warm")
            nc.gpsimd.memset(warm_in[:], 0.0)
            warm_ps = psp.tile([128, 512], f32, tag="ps")
            for _ in range(36):
                nc.tensor.matmul(warm_ps[:, :128], warm_in[:, :128],
                                 warm_in[:, :128], start=True, stop=True)

            x_tiles = [None] * NUM_TIME_STEPS
            w_tiles = [None] * NUM_TIME_STEPS

            def load_step(o, eng):
                if o >= NUM_TIME_STEPS:
                    return
                n = len(_band(o))
                k0 = int(_KSTART[o])
                w_t = wp.tile([128, TRI_BLOCK * 128], f16, tag="w")
                eng.dma_start(w_t[:, : n * 128],
                              wt_d[:, k0 * 128: (k0 + n) * 128])
                w_tiles[o] = w_t
                t = xp.tile([128, BC], f16, tag="x")
                if o == 0:
                    # split the first block so the very first matmuls can
                    # start as soon as 256 KB have landed; weight and x
                    # issue from different engines in parallel
                    nc.sync.dma_start(t[:, :512], xT_d[:, :512])
                    nc.sync.dma_start(t[:, 512:BC], xT_d[:, 512:BC])
                else:
                    eng.dma_start(t[:], xT_d[:, o * BC:(o + 1) * BC])
                x_tiles[o] = t

            load_step(0, nc.scalar)
            load_step(1, nc.sync)
            for o in range(2, NUM_TIME_STEPS):
                load_step(o, nc.scalar)

            for o in range(NUM_TIME_STEPS):
                band = list(_band(o))
                n = len(band)
                w_t = w_tiles[o]
                out_t = op.tile([128, BC], f32, tag="o")
                for h in range(NH):
                    ps = psp.tile([128, 512], f32, tag="ps")
                    for idx, j in enumerate(band):
                        nc.tensor.matmul(
                            ps[:],
                            w_t[:, idx * 128: (idx + 1) * 128],
                            x_tiles[j][:, h * 512: (h + 1) * 512],
                            start=(idx == 0),
                            stop=(idx == n - 1),
                        )
                    nc.vector.tensor_scalar_add(
                        out_t[:, h * 512: (h + 1) * 512], ps[:],
                        bias_t[:, o: o + 1])
                    nc.sync.dma_start(
                        yT_d[o, :, h * 512: (h + 1) * 512],
                        out_t[:, h * 512: (h + 1) * 512])

    nc.compile()
    return nc


def _get_program():
    global _PROGRAM
    if _PROGRAM is None:
        _PROGRAM = _build_program()
    return _PROGRAM


def _pack_inputs(x, weight, bias, mask):
    x = np.asarray(x, dtype=np.float32)
    weight = np.asarray(weight, dtype=np.float32)
    bias = np.asarray(bias, dtype=np.float32)
    mask = np.asarray(mask)

    wt_flat = np.empty((128, N_BLOCKS * 128), dtype=np.float16)
    k = 0
    for o in range(NUM_TIME_STEPS):
        for j in _band(o):
            blk = weight[o * 128:(o + 1) * 128, j * 128:(j + 1) * 128]
            mblk = mask[o * 128:(o + 1) * 128, j * 128:(j + 1) * 128]
            wt_flat[:, k * 128:(k + 1) * 128] = (blk * mblk).T
            k += 1

    bias_t = np.ascontiguousarray(bias.reshape(NUM_TIME_STEPS, 128).T)

    x16 = x.astype(np.float16)
    in_maps = []
    for c in range(N_CORES):
        xc = x16[c * BC:(c + 1) * BC]  # [BC, 4096]
        # -> [128 partitions(i within blk), NT * BC] fp16, contiguous rows
        xTc = np.ascontiguousarray(
            xc.reshape(BC, NUM_TIME_STEPS, 128).transpose(2, 1, 0)
        ).reshape(128, NUM_TIME_STEPS * BC)
        in_maps.append({
            "xT": xTc,
            "wt": wt_flat,
            "bias_t": bias_t,
        })
    return in_maps


def _run(inputs, trace=False):
    from concourse.bass_utils import run_bass_kernel_spmd

    nc = _get_program()
    in_maps = _pack_inputs(**inputs)
    res = run_bass_kernel_spmd(nc, in_maps, list(range(N_CORES)), trace=trace)

    y = np.empty((BATCH, OUT_SIZE), dtype=np.float32)
    for c in range(N_CORES):
        yTc = res.results[c]["yT"].reshape(OUT_SIZE, BC)
        y[c * BC:(c + 1) * BC] = yTc.T
    return y, res


def kernel(x, weight, bias, mask):
    y, _ = _run({"x": x, "weight": weight, "bias": bias, "mask": mask})
    return y
